# revision 20
# baseline (speedup 1.0000x reference)
"""NVFP4 block-scaled matmul (A @ B^T + bias) on 8 TRN2 NeuronCores.

Tensor-parallel over N: core i computes out[:, i*2048:(i+1)*2048].

Shipped mode ("prescaled"): the host marshals B to dequantized bf16
(fp4 decode + per-block scale, exact ops on exactly-representable
values), pre-transposed k-major [K, N/8] per core and packed 4 k-chunks
per DMA row-block; A (64x smaller) is fully dequantized to bf16 [K, M]
with the global scales folded in; bias replicated to [128, N/8].
Device kernel per core: stream 16 x 2MB loads (ACT-kicked HWDGE queue,
separate from the A/bias/store SP queue); 24 dummy matmuls pre-warm the
PE clock (HAM ungates to 2.4GHz after ~4us sustained) while the first
chunks stream in, discarded by chunk 0's start=True PSUM reset; 512 PE
matmuls accumulate 8 [128,512] f32 PSUM banks over 64 k-chunks; DVE
bias-add + bf16 store. Measured ~158us/exec steady-state: PE-bound
(self-loading bf16 matmuls cost ~330ns each incl. the per-matmul
LDWEIGHTS, measured 170us PE-only; DMA-only floor is ~111us at
~325GB/s/core).

Alternate device-side dequant modes kept for reference: "dequant"
(e4m3 value bytes on the wire at half the DMA bytes; ACT fp8->bf16 +
DVE scale-multiply with scales replicated across partitions by
log-doubling SBUF DMAs) measured ~2x slower on HW -- the replication
chain and extra engine passes dominate. "noscale"/"dmaonly"/"peonly"/
"prescaled_ct" are perf diagnostics.
"""

import numpy as np
import ml_dtypes

import concourse.bass as bass
import concourse.mybir as mybir
import concourse.tile as tile
from concourse import bacc
from concourse import bass_utils

P = 128
M, N, K = 256, 16384, 8192
NCORES = 8
NB = N // NCORES          # 2048  per-core N slab
KCH = K // P              # 64    k-chunks of 128
BLOCK = 16                # NVFP4 block size

_FP4 = np.array([0.0, 0.5, 1.0, 1.5, 2.0, 3.0, 4.0, 6.0,
                 -0.0, -0.5, -1.0, -1.5, -2.0, -3.0, -4.0, -6.0], np.float32)

# Host-prescaled bf16 wire format for B (2 bytes/elem)
PRESCALED_MODES = ("prescaled", "prescaled_ct", "prescaled_w", "dmaonly", "peonly")
# Host-prescaled fp8 e3m4 wire format for B (1 byte/elem, 4 mantissa bits).
# "e3": A stays bf16 (mixed-dtype matmul); "e3a": A also e3m4.
E3_MODES = ("e3", "e3a")
STREAM_MODES = PRESCALED_MODES + E3_MODES


def _cpl(mode, kch):
    # chunks packed side-by-side per DMA row-block: 2MB transfers either way
    return min(8, kch) if mode in E3_MODES else min(4, kch)


def _codes(x_int32: np.ndarray) -> np.ndarray:
    """[rows, K//2] int32 byte values -> [rows, K] uint8 fp4 codes
    (low nibble first, matching the reference)."""
    b = x_int32.astype(np.uint8)
    lo = b & 0xF
    hi = b >> 4
    return np.stack([lo, hi], axis=-1).reshape(b.shape[0], -1)


def permute_scale_rows(sbt: np.ndarray, kch: int) -> np.ndarray:
    """Reorder scale rows for the grouped on-chip replication: within each
    group of G chunks (8*G rows), original row 8*j + pd is stored at
    pd*G + j."""
    G = min(8, kch)
    rows, n = sbt.shape
    return np.ascontiguousarray(
        sbt.reshape(-1, G, 8, n).transpose(0, 2, 1, 3).reshape(rows, n)
    )


def pack_chunks(btf: np.ndarray, kch: int) -> np.ndarray:
    """[kch*P, n] -> [kch//cpl*P, cpl*n]: cpl chunks side by side."""
    cpl = min(4, kch)
    n = btf.shape[1]
    return np.ascontiguousarray(
        btf.reshape(kch // cpl, cpl, P, n).transpose(0, 2, 1, 3)
    ).reshape(kch // cpl * P, cpl * n)


def k_perm(kch: int) -> np.ndarray:
    """Row permutation applied on host: partition p of chunk c holds
    original k-row c*128 + (p % 8)*16 + p//8."""
    p = np.arange(P)
    within = (p % 8) * 16 + p // 8
    return (np.arange(kch)[:, None] * P + within[None, :]).reshape(-1)


def tile_body(tc, out_ap, at_ap, bt_ap, sbt_ap, bias_ap, *, kch=KCH, nb=NB, m=M,
              repeat=1, mode="dequant", gs_ap=None):
    """Per-core kernel body. Shapes:
      at_ap  [kch*128, m]   bf16   A' transposed (dequant, k-major)
      bt_ap  [kch*128, nb]  uint8  e4m3 value bytes of B, k-major
      sbt_ap [kch*8,  nb]   bf16   b_scale transposed (kb-major)
      bias_ap [128, nb]     bf16   bias slab replicated across partitions
      out_ap [m, nb]        bf16
    """
    nc = tc.nc
    assert m % P == 0
    mh = m // P               # m subtiles (2)
    nq = nb // 512            # psum-width quarters (4)
    srows = kch * 8           # total scale rows
    sp = min(srows, P)        # scale slab partition dim
    so = srows // sp

    with (
        tc.tile_pool(name="const", bufs=1) as const,
        tc.tile_pool(name="bv", bufs=6) as bv_pool,
        tc.tile_pool(name="srep", bufs=2) as srep_pool,
        tc.tile_pool(name="bp", bufs=4) as bp_pool,
        tc.tile_pool(name="psum", bufs=1, space="PSUM") as psum_pool,
        tc.tile_pool(name="outp", bufs=2) as out_pool,
    ):
        # Resident tensors (A loaded in 4 pieces so chunk 0 isn't gated on
        # the whole 4MB transfer)
        a_dt = mybir.dt.uint8 if mode == "e3a" else mybir.dt.bfloat16
        a_sb = const.tile([P, kch, m], a_dt, name="a_sb")
        at3 = at_ap.rearrange("(c p) m -> p c m", p=P)
        a_step = max(1, kch // 4)
        for c0 in range(0, kch, a_step):
            c1 = min(kch, c0 + a_step)
            nc.sync.dma_start(a_sb[:, c0:c1], at3[:, c0:c1])
        if mode in STREAM_MODES:
            s_sb = None
        else:
            s_sb = const.tile([sp, so, nb], mybir.dt.bfloat16, name="s_sb")
            nc.sync.dma_start(s_sb, sbt_ap.rearrange("(o p) n -> p o n", p=sp))
        bias_sb = const.tile([P, nb], mybir.dt.bfloat16, name="bias_sb")
        nc.sync.dma_start(bias_sb, bias_ap)
        if gs_ap is not None:
            gs_sb = const.tile([P, 1], mybir.dt.float32, name="gs_sb")
            nc.sync.dma_start(gs_sb, gs_ap)
        else:
            gs_sb = None

        mh = m // P
        nq = nb // 512
        if mode == "prescaled_w":
            psums_w = [
                psum_pool.tile([P, nb], mybir.dt.float32, name=f"psw_{h}")
                for h in range(mh)
            ]
            psums = [psums_w[h][:, q * 512:(q + 1) * 512]
                     for h in range(mh) for q in range(nq)]
        else:
            psums_w = None
            psums = [
                psum_pool.tile([P, 512], mybir.dt.float32, name=f"ps_{h}_{q}")
                for h in range(mh) for q in range(nq)
            ]

        # PE clock warmup: ~5us of dummy matmuls while the first B chunks
        # stream in, so the real stream starts at 2.4GHz (HAM ungates after
        # ~3.4us of sustained PE activity). Chunk 0's start=True resets PSUM,
        # discarding these. Outside the repeat loop: in steady state the PE
        # is already warm and these would be pure per-iteration overhead.
        if mode in ("prescaled", "e3", "e3a"):
            warm = const.tile([P, 512], mybir.dt.bfloat16, name="warm")
            nc.vector.memset(warm, 0.0)
            for w in range(24):
                wi = nc.tensor.matmul(psums[0], lhsT=warm[:, :128], rhs=warm,
                                      start=True, stop=True)
                if w > 0:
                    wi.ins.ldweights = False

        def body():
            _pipeline(tc, out_ap, bt_ap, a_sb, s_sb, bias_sb, psums, psums_w,
                      kch=kch, nb=nb, m=m, sp=sp, mode=mode, gs_sb=gs_sb,
                      bv_pool=bv_pool, srep_pool=srep_pool, bp_pool=bp_pool,
                      out_pool=out_pool)

        if repeat == 1:
            body()
        else:
            with tc.For_i(0, repeat, 1,
                          hint_engines=(mybir.EngineType.PE,
                                        mybir.EngineType.Activation,
                                        mybir.EngineType.DVE,
                                        mybir.EngineType.Pool,
                                        mybir.EngineType.SP)):
                body()


def _pipeline(tc, out_ap, bt_ap, a_sb, s_sb, bias_sb, psums, psums_w, *,
              kch, nb, m, sp,
              bv_pool, srep_pool, bp_pool, out_pool,
              mode="dequant", gs_sb=None):
        nc = tc.nc
        mh = m // P
        nq = nb // 512

        # host packs CPL k-chunks side by side in the free dim:
        # bt row-block l holds chunks l*CPL..(l+1)*CPL at column offsets j*nb
        cpl = _cpl(mode, kch)
        bt3 = bt_ap.rearrange("(l p) n -> l p n", p=P)

        if mode in STREAM_MODES:
            # bt is host-prescaled (bf16 or e3m4); pure DMA + matmul + bias.
            # dmaonly/peonly are perf diagnostics (wrong results).
            e3 = mode in E3_MODES
            bv_dt = mybir.dt.uint8 if e3 else mybir.dt.bfloat16
            if mode == "e3a":
                a_mm = a_sb.bitcast(mybir.dt.float8e3)
            else:
                a_mm = a_sb
            bv0 = None
            for l in range(kch // cpl):
                if mode == "peonly" and bv0 is not None:
                    bv = bv0
                else:
                    bv = bv_pool.tile([P, cpl * nb], bv_dt, name="bv")
                    # B loads ride the ACT-kicked HWDGE queue; the A/bias
                    # prologue and output stores stay on the SP queue
                    nc.scalar.dma_start(bv, bt3[l])
                    bv0 = bv
                bvm = bv.bitcast(mybir.dt.float8e3) if e3 else bv
                for j in range(cpl):
                    c = l * cpl + j
                    first, last = c == 0, c == kch - 1
                    if mode == "dmaonly" and not (first or last):
                        continue
                    if mode == "prescaled_w":
                        for h in range(mh):
                            nc.tensor.matmul(
                                psums_w[h],
                                lhsT=a_mm[:, c, h * P:(h + 1) * P],
                                rhs=bvm[:, j * nb:(j + 1) * nb],
                                start=first,
                                stop=last,
                            )
                        continue
                    for h in range(mh):
                        for q in range(nq):
                            rhs = bvm[:, j * nb + q * 512: j * nb + (q + 1) * 512]
                            if mode == "prescaled_ct":
                                # column-tiled: 4 concurrent 32-col-band
                                # matmuls in separate array quadrants; the
                                # small band weight loads can pull ahead.
                                for b in range(4):
                                    bs = slice(b * 32, (b + 1) * 32)
                                    nc.tensor.matmul(
                                        psums[h * nq + q][bs, :],
                                        lhsT=a_sb[:, c, h * P + b * 32:
                                                  h * P + (b + 1) * 32],
                                        rhs=rhs,
                                        start=first,
                                        stop=last,
                                        tile_position=(0, b * 32),
                                    )
                            else:
                                mi = nc.tensor.matmul(
                                    psums[h * nq + q],
                                    lhsT=a_mm[:, c, h * P:(h + 1) * P],
                                    rhs=rhs,
                                    start=first,
                                    stop=last,
                                )
                                if q > 0:
                                    # q=0 loaded this (c, h) weight tile; the
                                    # other 3 psum quarters reuse it. Walrus
                                    # otherwise emits a ~108ns LDWEIGHTS per
                                    # matmul that row-group-conflicts with the
                                    # in-flight stream (+47ns/matmul).
                                    mi.ins.ldweights = False
            _epilogue(nc, out_ap, bias_sb, psums, out_pool, mh, nq, gs_sb)
            return

        G = min(8, kch)            # chunks per scale-replication group
        for g in range(kch // G):
            # One seed DMA + 4 doubling hops replicate the scales for G
            # chunks at once. Host stores scale rows so that s_sb row
            # pd*G + j (within the group's G*8 rows) = scale row of
            # chunk g*G+j, sub-row pd; after doubling, partition p of
            # column-block j holds scale row 8*(g*G+j) + (p % 8).
            if mode == "noscale":
                srep = None
            else:
                p0 = (8 * G * g) % sp
                o0 = (8 * G * g) // sp
                srep = srep_pool.tile([P, G * nb], mybir.dt.bfloat16, name="srep")
                nc.sync.dma_start(srep[0:8, :], s_sb[p0:p0 + 8 * G, o0, :])
                w = 8
                while w < P:
                    nc.sync.dma_start(srep[w:2 * w], srep[0:w])
                    w *= 2

            for lj in range(G // cpl):
                l = (g * G) // cpl + lj
                # raw e4m3 value bytes for cpl k-chunks in one DMA
                bv = bv_pool.tile([P, cpl * nb], mybir.dt.uint8, name="bv")
                nc.sync.dma_start(bv, bt3[l])
                bv8 = bv.bitcast(mybir.dt.float8e4)

                for jc in range(cpl):
                    c = l * cpl + jc
                    j = c - g * G      # chunk index within the srep group
                    # fp8 -> bf16 convert + scale, produced in 512-wide
                    # quarters so PE starts on quarter 0 while later
                    # quarters convert. ACT ~2.4x GpSimd rate: 3:1 split.
                    bp = bp_pool.tile([P, nb], mybir.dt.bfloat16, name="bp")
                    for q in range(nq):
                        ql = slice(q * 512, (q + 1) * 512)
                        bl = slice(jc * nb + q * 512, jc * nb + (q + 1) * 512)
                        if q == nq - 1:
                            nc.gpsimd.tensor_copy(bp[:, ql], bv8[:, bl])
                        else:
                            nc.scalar.copy(bp[:, ql], bv8[:, bl])
                        s_in = (s_sb[:, 0, q * 512:(q + 1) * 512] if srep is None
                                else srep[:, j * nb + q * 512: j * nb + (q + 1) * 512])
                        nc.vector.tensor_mul(out=bp[:, ql], in0=bp[:, ql], in1=s_in)

                    first = c == 0
                    last = c == kch - 1
                    for h in range(mh):
                        for q in range(nq):
                            nc.tensor.matmul(
                                psums[h * nq + q],
                                lhsT=a_sb[:, c, h * P:(h + 1) * P],
                                rhs=bp[:, q * 512:(q + 1) * 512],
                                start=first,
                                stop=last,
                            )

        _epilogue(nc, out_ap, bias_sb, psums, out_pool, mh, nq)


def _epilogue(nc, out_ap, bias_sb, psums, out_pool, mh, nq, gs_sb=None):
    """(optional global-scale multiply +) bias add + cast to bf16 + store.

    Banks evacuated in the same (h, q) order the next iteration's chunk-0
    matmuls will claim them, so the PE restarts after one bank's drain."""
    for h in range(mh):
        for q in range(nq):
            ot = out_pool.tile([P, 512], mybir.dt.bfloat16, name="ot")
            if gs_sb is None:
                nc.vector.tensor_add(
                    out=ot,
                    in0=psums[h * nq + q],
                    in1=bias_sb[:, q * 512:(q + 1) * 512],
                )
            else:
                nc.vector.scalar_tensor_tensor(
                    out=ot,
                    in0=psums[h * nq + q],
                    scalar=gs_sb[:, 0:1],
                    in1=bias_sb[:, q * 512:(q + 1) * 512],
                    op0=mybir.AluOpType.mult,
                    op1=mybir.AluOpType.add,
                )
            nc.sync.dma_start(
                out_ap[h * P:(h + 1) * P, q * 512:(q + 1) * 512], ot
            )


def build(kch=KCH, nb=NB, m=M, repeat=1, mode="dequant"):
    nc = bacc.Bacc(
        "TRN2",
        target_bir_lowering=False,
        debug=False,
        num_devices=NCORES,
    )
    bt_dt = (mybir.dt.bfloat16 if mode in PRESCALED_MODES else mybir.dt.uint8)
    cpl = _cpl(mode, kch)
    at_dt = mybir.dt.uint8 if mode == "e3a" else mybir.dt.bfloat16
    at = nc.dram_tensor("at", [kch * P, m], at_dt, kind="ExternalInput").ap()
    bt = nc.dram_tensor("bt", [kch * P // cpl, cpl * nb], bt_dt, kind="ExternalInput").ap()
    sbt = nc.dram_tensor("sbt", [kch * 8, nb], mybir.dt.bfloat16, kind="ExternalInput").ap()
    bias = nc.dram_tensor("bias", [P, nb], mybir.dt.bfloat16, kind="ExternalInput").ap()
    gs = (nc.dram_tensor("gs", [P, 1], mybir.dt.float32, kind="ExternalInput").ap()
          if mode == "e3a" else None)
    out = nc.dram_tensor("out", [m, nb], mybir.dt.bfloat16, kind="ExternalOutput").ap()
    with tile.TileContext(nc) as tc:
        tile_body(tc, out, at, bt, sbt, bias, kch=kch, nb=nb, m=m, repeat=repeat,
                  mode=mode, gs_ap=gs)
    nc.compile()
    return nc


def marshal(a, a_scale, a_global_scale, b, b_scale, b_global_scale, bias,
            mode="dequant"):
    """Host-side input prep. Returns per-core in_maps."""
    a = np.asarray(a)
    a_scale = np.asarray(a_scale, np.float32)
    ga = float(np.asarray(a_global_scale, np.float32))
    b = np.asarray(b)
    b_scale = np.asarray(b_scale, np.float32)
    gb = float(np.asarray(b_global_scale, np.float32))
    bias = np.asarray(bias, np.float32)

    # A side: full dequant (small), fold global scales, transpose to [K, M].
    # In e3 modes B is prescaled by 2x (keeps all e3m4 values normal; min
    # nonzero |bval*bs| = 0.125 < 2^-2 = e3m4 min normal), compensated here.
    a_vals = _FP4[_codes(a)]                                   # [M, K]
    if mode == "e3a":
        # A also e3m4: scale by 2 to stay normal; global scales move to a
        # runtime epilogue multiplier gs = ga*gb/4.
        a_deq = a_vals.reshape(M, K // BLOCK, BLOCK) * (2.0 * a_scale)[..., None]
        at = np.ascontiguousarray(a_deq.reshape(M, K).T).astype(
            ml_dtypes.float8_e3m4).view(np.uint8)
    else:
        a_gl = ga * gb / (2.0 if mode in E3_MODES else 1.0)
        a_deq = a_vals.reshape(M, K // BLOCK, BLOCK) * (a_scale * a_gl)[..., None]
        at = np.ascontiguousarray(a_deq.reshape(M, K).T).astype(ml_dtypes.bfloat16)

    # B side: decode codes to e4m3 value bytes, transpose to [K, N]
    if mode in PRESCALED_MODES:
        bv = _FP4[_codes(b)].reshape(N, K // BLOCK, BLOCK)
        bv = (bv * b_scale.astype(np.float32)[..., None]).reshape(N, K)
        btf = np.ascontiguousarray(bv.T).astype(ml_dtypes.bfloat16)  # [K, N]
    elif mode in E3_MODES:
        bv = _FP4[_codes(b)].reshape(N, K // BLOCK, BLOCK)
        bv = (bv * (2.0 * b_scale.astype(np.float32))[..., None]).reshape(N, K)
        btf = np.ascontiguousarray(bv.T).astype(ml_dtypes.float8_e3m4).view(np.uint8)
    else:
        b_vals_e4m3 = _FP4.astype(ml_dtypes.float8_e4m3)[_codes(b)]  # [N, K]
        btf = np.ascontiguousarray(b_vals_e4m3.T).view(np.uint8)     # [K, N] u8

    # within-chunk k-row permutation: partition p holds original row
    # (p % 8) * 16 + p // 8, so its scale row is (p % 8)
    perm = k_perm(K // P)
    at = np.ascontiguousarray(at[perm])
    btf = btf[perm]
    # pack cpl chunks side by side in the free dim (one DMA per cpl chunks)
    kch = K // P
    cpl = _cpl(mode, kch)
    nfull = btf.shape[1]
    btf = btf.reshape(kch // cpl, cpl, P, nfull).transpose(0, 2, 1, 3)
    sbt_f = np.ascontiguousarray(b_scale.T).astype(ml_dtypes.bfloat16)  # [K/16, N]
    sbt_f = permute_scale_rows(sbt_f, K // P)

    in_maps = []
    for ci in range(NCORES):
        sl = slice(ci * NB, (ci + 1) * NB)
        bias_rep = np.ascontiguousarray(
            np.broadcast_to(bias[None, sl], (P, NB))
        ).astype(ml_dtypes.bfloat16)
        bt_core = np.ascontiguousarray(btf[..., sl]).reshape(
            kch // cpl * P, cpl * NB)
        im = {
            "at": at,
            "bt": bt_core,
            "sbt": np.ascontiguousarray(sbt_f[:, sl]),
            "bias": bias_rep,
        }
        if mode == "e3a":
            im["gs"] = np.full((P, 1), ga * gb / 4.0, np.float32)
        in_maps.append(im)
    return in_maps


_CACHE = {}


MODE = "e3"


def kernel(a, a_scale, a_global_scale, b, b_scale, b_global_scale, bias):
    in_maps = marshal(a, a_scale, a_global_scale, b, b_scale, b_global_scale,
                      bias, mode=MODE)
    if "nc" not in _CACHE:
        _CACHE["nc"] = build(mode=MODE)
    res = bass_utils.run_bass_kernel_spmd(
        _CACHE["nc"], in_maps, core_ids=list(range(NCORES))
    )
    return np.concatenate([r["out"] for r in res.results], axis=1)



# revision 33
# speedup vs baseline: 1.3604x; 1.3604x over previous
"""NVFP4 block-scaled matmul (A @ B^T + bias) on 8 TRN2 NeuronCores.

Tensor-parallel over N: core i computes out[:, i*2048:(i+1)*2048].

Shipped mode ("prescaled"): the host marshals B to dequantized bf16
(fp4 decode + per-block scale, exact ops on exactly-representable
values), pre-transposed k-major [K, N/8] per core and packed 4 k-chunks
per DMA row-block; A (64x smaller) is fully dequantized to bf16 [K, M]
with the global scales folded in; bias replicated to [128, N/8].
Device kernel per core: stream 16 x 2MB loads (ACT-kicked HWDGE queue,
separate from the A/bias/store SP queue); 24 dummy matmuls pre-warm the
PE clock (HAM ungates to 2.4GHz after ~4us sustained) while the first
chunks stream in, discarded by chunk 0's start=True PSUM reset; 512 PE
matmuls accumulate 8 [128,512] f32 PSUM banks over 64 k-chunks; DVE
bias-add + bf16 store. Measured ~158us/exec steady-state: PE-bound
(self-loading bf16 matmuls cost ~330ns each incl. the per-matmul
LDWEIGHTS, measured 170us PE-only; DMA-only floor is ~111us at
~325GB/s/core).

Alternate device-side dequant modes kept for reference: "dequant"
(e4m3 value bytes on the wire at half the DMA bytes; ACT fp8->bf16 +
DVE scale-multiply with scales replicated across partitions by
log-doubling SBUF DMAs) measured ~2x slower on HW -- the replication
chain and extra engine passes dominate. "noscale"/"dmaonly"/"peonly"/
"prescaled_ct" are perf diagnostics.
"""

import numpy as np
import ml_dtypes

import concourse.bass as bass
import concourse.mybir as mybir
import concourse.tile as tile
from concourse import bacc
from concourse import bass_utils

P = 128
M, N, K = 256, 16384, 8192
NCORES = 8
NB = N // NCORES          # 2048  per-core N slab
KCH = K // P              # 64    k-chunks of 128
BLOCK = 16                # NVFP4 block size

_FP4 = np.array([0.0, 0.5, 1.0, 1.5, 2.0, 3.0, 4.0, 6.0,
                 -0.0, -0.5, -1.0, -1.5, -2.0, -3.0, -4.0, -6.0], np.float32)

# Host-prescaled bf16 wire format for B (2 bytes/elem)
PRESCALED_MODES = ("prescaled", "prescaled_ct", "prescaled_w", "dmaonly", "peonly")
# Host-prescaled fp8 e3m4 wire format for B (1 byte/elem, 4 mantissa bits).
# "e3": A stays bf16 (mixed-dtype matmul); "e3a": A also e3m4.
E3_MODES = ("e3", "e3a")
STREAM_MODES = PRESCALED_MODES + E3_MODES

# "hyb": the first HYB_NDR chunks of 256 k-values run as fp8e4 DoubleRow
# matmuls (2 fp8 MACs/cell/cycle -> ~2x column rate), the remaining
# 64-2*HYB_NDR chunks of 128 as bf16 x e3m4 regular matmuls. e4m3 has only
# 3 mantissa bits, so the DR slice costs ~2.8% relative error at f=1;
# splitting K keeps total error sqrt(f*2.81^2 + (1-f)*1.02^2) under the
# 2e-2 gate while recovering most of the DoubleRow speed.
HYB_NDR = 12
CPLD = 4          # DR chunks (256 k each) packed per DMA row-block (2MB)


def _cpl(mode, kch):
    # chunks packed side-by-side per DMA row-block: 2MB transfers either way
    return min(8, kch) if mode in E3_MODES else min(4, kch)


def _codes(x_int32: np.ndarray) -> np.ndarray:
    """[rows, K//2] int32 byte values -> [rows, K] uint8 fp4 codes
    (low nibble first, matching the reference)."""
    b = x_int32.astype(np.uint8)
    lo = b & 0xF
    hi = b >> 4
    return np.stack([lo, hi], axis=-1).reshape(b.shape[0], -1)


def permute_scale_rows(sbt: np.ndarray, kch: int) -> np.ndarray:
    """Reorder scale rows for the grouped on-chip replication: within each
    group of G chunks (8*G rows), original row 8*j + pd is stored at
    pd*G + j."""
    G = min(8, kch)
    rows, n = sbt.shape
    return np.ascontiguousarray(
        sbt.reshape(-1, G, 8, n).transpose(0, 2, 1, 3).reshape(rows, n)
    )


def pack_chunks(btf: np.ndarray, kch: int) -> np.ndarray:
    """[kch*P, n] -> [kch//cpl*P, cpl*n]: cpl chunks side by side."""
    cpl = min(4, kch)
    n = btf.shape[1]
    return np.ascontiguousarray(
        btf.reshape(kch // cpl, cpl, P, n).transpose(0, 2, 1, 3)
    ).reshape(kch // cpl * P, cpl * n)


def k_perm(kch: int) -> np.ndarray:
    """Row permutation applied on host: partition p of chunk c holds
    original k-row c*128 + (p % 8)*16 + p//8."""
    p = np.arange(P)
    within = (p % 8) * 16 + p // 8
    return (np.arange(kch)[:, None] * P + within[None, :]).reshape(-1)


def tile_body(tc, out_ap, at_ap, bt_ap, sbt_ap, bias_ap, *, kch=KCH, nb=NB, m=M,
              repeat=1, mode="dequant", gs_ap=None):
    """Per-core kernel body. Shapes:
      at_ap  [kch*128, m]   bf16   A' transposed (dequant, k-major)
      bt_ap  [kch*128, nb]  uint8  e4m3 value bytes of B, k-major
      sbt_ap [kch*8,  nb]   bf16   b_scale transposed (kb-major)
      bias_ap [128, nb]     bf16   bias slab replicated across partitions
      out_ap [m, nb]        bf16
    """
    nc = tc.nc
    assert m % P == 0
    mh = m // P               # m subtiles (2)
    nq = nb // 512            # psum-width quarters (4)
    srows = kch * 8           # total scale rows
    sp = min(srows, P)        # scale slab partition dim
    so = srows // sp

    with (
        tc.tile_pool(name="const", bufs=1) as const,
        tc.tile_pool(name="bv", bufs=6) as bv_pool,
        tc.tile_pool(name="srep", bufs=2) as srep_pool,
        tc.tile_pool(name="bp", bufs=4) as bp_pool,
        tc.tile_pool(name="psum", bufs=1, space="PSUM") as psum_pool,
        tc.tile_pool(name="outp", bufs=8) as out_pool,
    ):
        # Resident tensors (A loaded in 4 pieces so chunk 0 isn't gated on
        # the whole 4MB transfer)
        a_dt = mybir.dt.uint8 if mode == "e3a" else mybir.dt.bfloat16
        a_sb = const.tile([P, kch, m], a_dt, name="a_sb")
        at3 = at_ap.rearrange("(c p) m -> p c m", p=P)
        a_step = max(1, kch // 4)
        for c0 in range(0, kch, a_step):
            c1 = min(kch, c0 + a_step)
            nc.sync.dma_start(a_sb[:, c0:c1], at3[:, c0:c1])
        if mode in STREAM_MODES:
            s_sb = None
        else:
            s_sb = const.tile([sp, so, nb], mybir.dt.bfloat16, name="s_sb")
            nc.sync.dma_start(s_sb, sbt_ap.rearrange("(o p) n -> p o n", p=sp))
        bias_sb = const.tile([P, nb], mybir.dt.bfloat16, name="bias_sb")
        nc.sync.dma_start(bias_sb, bias_ap)
        if gs_ap is not None:
            gs_sb = const.tile([P, 1], mybir.dt.float32, name="gs_sb")
            nc.sync.dma_start(gs_sb, gs_ap)
        else:
            gs_sb = None

        mh = m // P
        nq = nb // 512
        if mode == "prescaled_w":
            psums_w = [
                psum_pool.tile([P, nb], mybir.dt.float32, name=f"psw_{h}")
                for h in range(mh)
            ]
            psums = [psums_w[h][:, q * 512:(q + 1) * 512]
                     for h in range(mh) for q in range(nq)]
        else:
            psums_w = None
            psums = [
                psum_pool.tile([P, 512], mybir.dt.float32, name=f"ps_{h}_{q}")
                for h in range(mh) for q in range(nq)
            ]

        # PE clock warmup: ~5us of dummy matmuls while the first B chunks
        # stream in, so the real stream starts at 2.4GHz (HAM ungates after
        # ~3.4us of sustained PE activity). Chunk 0's start=True resets PSUM,
        # discarding these. Outside the repeat loop: in steady state the PE
        # is already warm and these would be pure per-iteration overhead.
        if mode in ("prescaled", "e3", "e3a"):
            warm = const.tile([P, 512], mybir.dt.bfloat16, name="warm")
            nc.vector.memset(warm, 0.0)
            for w in range(24):
                wi = nc.tensor.matmul(psums[0], lhsT=warm[:, :128], rhs=warm,
                                      start=True, stop=True)
                if w > 0:
                    wi.ins.ldweights = False

        def body():
            _pipeline(tc, out_ap, bt_ap, a_sb, s_sb, bias_sb, psums, psums_w,
                      kch=kch, nb=nb, m=m, sp=sp, mode=mode, gs_sb=gs_sb,
                      bv_pool=bv_pool, srep_pool=srep_pool, bp_pool=bp_pool,
                      out_pool=out_pool)

        if repeat == 1:
            body()
        else:
            with tc.For_i(0, repeat, 1,
                          hint_engines=(mybir.EngineType.PE,
                                        mybir.EngineType.Activation,
                                        mybir.EngineType.DVE,
                                        mybir.EngineType.Pool,
                                        mybir.EngineType.SP)):
                body()


def _pipeline(tc, out_ap, bt_ap, a_sb, s_sb, bias_sb, psums, psums_w, *,
              kch, nb, m, sp,
              bv_pool, srep_pool, bp_pool, out_pool,
              mode="dequant", gs_sb=None):
        nc = tc.nc
        mh = m // P
        nq = nb // 512

        # host packs CPL k-chunks side by side in the free dim:
        # bt row-block l holds chunks l*CPL..(l+1)*CPL at column offsets j*nb
        cpl = _cpl(mode, kch)
        bt3 = bt_ap.rearrange("(l p) n -> l p n", p=P)

        if mode in STREAM_MODES:
            # bt is host-prescaled (bf16 or e3m4); pure DMA + matmul + bias.
            # dmaonly/peonly are perf diagnostics (wrong results).
            e3 = mode in E3_MODES
            bv_dt = mybir.dt.uint8 if e3 else mybir.dt.bfloat16
            if mode == "e3a":
                a_mm = a_sb.bitcast(mybir.dt.float8e3)
            else:
                a_mm = a_sb
            bv0 = None
            for l in range(kch // cpl):
                if mode == "peonly" and bv0 is not None:
                    bv = bv0
                else:
                    bv = bv_pool.tile([P, cpl * nb], bv_dt, name="bv")
                    # B loads ride the ACT-kicked HWDGE queue; the A/bias
                    # prologue stays on the SP queue. Block 0's first chunk
                    # gets its own small kick so the post-barrier matmul
                    # restart waits ~1us instead of a full 2MB transfer.
                    if l == 0:
                        nc.scalar.dma_start(bv[:, :nb], bt3[l][:, :nb])
                        nc.scalar.dma_start(bv[:, nb:], bt3[l][:, nb:])
                    else:
                        nc.scalar.dma_start(bv, bt3[l])
                    bv0 = bv
                bvm = bv.bitcast(mybir.dt.float8e3) if e3 else bv
                for j in range(cpl):
                    c = l * cpl + j
                    first, last = c == 0, c == kch - 1
                    if mode == "dmaonly" and not (first or last):
                        continue
                    if mode == "prescaled_w":
                        for h in range(mh):
                            nc.tensor.matmul(
                                psums_w[h],
                                lhsT=a_mm[:, c, h * P:(h + 1) * P],
                                rhs=bvm[:, j * nb:(j + 1) * nb],
                                start=first,
                                stop=last,
                            )
                        continue
                    for h in range(mh):
                        for q in range(nq):
                            rhs = bvm[:, j * nb + q * 512: j * nb + (q + 1) * 512]
                            if mode == "prescaled_ct":
                                # column-tiled: 4 concurrent 32-col-band
                                # matmuls in separate array quadrants; the
                                # small band weight loads can pull ahead.
                                for b in range(4):
                                    bs = slice(b * 32, (b + 1) * 32)
                                    nc.tensor.matmul(
                                        psums[h * nq + q][bs, :],
                                        lhsT=a_sb[:, c, h * P + b * 32:
                                                  h * P + (b + 1) * 32],
                                        rhs=rhs,
                                        start=first,
                                        stop=last,
                                        tile_position=(0, b * 32),
                                    )
                            else:
                                mi = nc.tensor.matmul(
                                    psums[h * nq + q],
                                    lhsT=a_mm[:, c, h * P:(h + 1) * P],
                                    rhs=rhs,
                                    start=first,
                                    stop=last,
                                )
                                if q > 0:
                                    # q=0 loaded this (c, h) weight tile; the
                                    # other 3 psum quarters reuse it. Walrus
                                    # otherwise emits a ~108ns LDWEIGHTS per
                                    # matmul that row-group-conflicts with the
                                    # in-flight stream (+47ns/matmul).
                                    mi.ins.ldweights = False
            _epilogue(nc, out_ap, bias_sb, psums, out_pool, mh, nq, gs_sb)
            return

        G = min(8, kch)            # chunks per scale-replication group
        for g in range(kch // G):
            # One seed DMA + 4 doubling hops replicate the scales for G
            # chunks at once. Host stores scale rows so that s_sb row
            # pd*G + j (within the group's G*8 rows) = scale row of
            # chunk g*G+j, sub-row pd; after doubling, partition p of
            # column-block j holds scale row 8*(g*G+j) + (p % 8).
            if mode == "noscale":
                srep = None
            else:
                p0 = (8 * G * g) % sp
                o0 = (8 * G * g) // sp
                srep = srep_pool.tile([P, G * nb], mybir.dt.bfloat16, name="srep")
                nc.sync.dma_start(srep[0:8, :], s_sb[p0:p0 + 8 * G, o0, :])
                w = 8
                while w < P:
                    nc.sync.dma_start(srep[w:2 * w], srep[0:w])
                    w *= 2

            for lj in range(G // cpl):
                l = (g * G) // cpl + lj
                # raw e4m3 value bytes for cpl k-chunks in one DMA
                bv = bv_pool.tile([P, cpl * nb], mybir.dt.uint8, name="bv")
                nc.sync.dma_start(bv, bt3[l])
                bv8 = bv.bitcast(mybir.dt.float8e4)

                for jc in range(cpl):
                    c = l * cpl + jc
                    j = c - g * G      # chunk index within the srep group
                    # fp8 -> bf16 convert + scale, produced in 512-wide
                    # quarters so PE starts on quarter 0 while later
                    # quarters convert. ACT ~2.4x GpSimd rate: 3:1 split.
                    bp = bp_pool.tile([P, nb], mybir.dt.bfloat16, name="bp")
                    for q in range(nq):
                        ql = slice(q * 512, (q + 1) * 512)
                        bl = slice(jc * nb + q * 512, jc * nb + (q + 1) * 512)
                        if q == nq - 1:
                            nc.gpsimd.tensor_copy(bp[:, ql], bv8[:, bl])
                        else:
                            nc.scalar.copy(bp[:, ql], bv8[:, bl])
                        s_in = (s_sb[:, 0, q * 512:(q + 1) * 512] if srep is None
                                else srep[:, j * nb + q * 512: j * nb + (q + 1) * 512])
                        nc.vector.tensor_mul(out=bp[:, ql], in0=bp[:, ql], in1=s_in)

                    first = c == 0
                    last = c == kch - 1
                    for h in range(mh):
                        for q in range(nq):
                            nc.tensor.matmul(
                                psums[h * nq + q],
                                lhsT=a_sb[:, c, h * P:(h + 1) * P],
                                rhs=bp[:, q * 512:(q + 1) * 512],
                                start=first,
                                stop=last,
                            )

        _epilogue(nc, out_ap, bias_sb, psums, out_pool, mh, nq)


def hyb_body(tc, out_ap, at_ap, bt_ap, adr_ap, btdr_ap, bias_ap, *,
             ndr, kreg, nb=NB, m=M, repeat=1):
    """K-split kernel: ndr DoubleRow chunks (K=256, both sides e4m3) then
    kreg regular chunks (K=128, bf16 A x e3m4 B), all accumulating into the
    same 8 PSUM banks; bias-add epilogue."""
    nc = tc.nc
    mh = m // P
    nq = nb // 512
    cpl = min(8, kreg)

    with (
        tc.tile_pool(name="const", bufs=1) as const,
        tc.tile_pool(name="bv", bufs=6) as bv_pool,
        tc.tile_pool(name="bvd", bufs=3) as bvd_pool,
        tc.tile_pool(name="psum", bufs=1, space="PSUM") as psum_pool,
        tc.tile_pool(name="outp", bufs=8) as out_pool,
    ):
        # resident A for both segments
        a_sb = const.tile([P, kreg, m], mybir.dt.bfloat16, name="a_sb")
        at3 = at_ap.rearrange("(c p) m -> p c m", p=P)
        a_step = max(1, kreg // 4)
        for c0 in range(0, kreg, a_step):
            c1 = min(kreg, c0 + a_step)
            nc.sync.dma_start(a_sb[:, c0:c1], at3[:, c0:c1])
        adr_sb = const.tile([P, ndr, 2, m], mybir.dt.uint8, name="adr_sb")
        nc.sync.dma_start(adr_sb, adr_ap.rearrange("(c i p) m -> p c i m",
                                                   p=P, i=2))
        adr_mm = adr_sb.bitcast(mybir.dt.float8e4)
        bias_sb = const.tile([P, nb], mybir.dt.bfloat16, name="bias_sb")
        nc.sync.dma_start(bias_sb, bias_ap)

        psum_all = psum_pool.tile([P, mh * nq * 512], mybir.dt.float32,
                                  name="ps_all")
        psums = [psum_all[:, b * 512:(b + 1) * 512] for b in range(mh * nq)]

        warm = const.tile([P, 512], mybir.dt.bfloat16, name="warm")
        nc.vector.memset(warm, 0.0)
        for _ in range(24):
            nc.tensor.matmul(psums[0], lhsT=warm[:, :128], rhs=warm,
                             start=True, stop=True)

        bt3 = bt_ap.rearrange("(l p) n -> l p n", p=P)
        btdr3 = btdr_ap.rearrange("(l p) n -> l p n", p=P)

        def body():
            # --- DoubleRow segment ---
            for l in range(ndr // CPLD):
                bvd = bvd_pool.tile([P, CPLD, 2, nb], mybir.dt.uint8,
                                    name="bvd")
                src = btdr3[l].rearrange("p (j i n) -> p j i n", j=CPLD, i=2)
                if l == 0:
                    # small first kick so the post-barrier restart waits
                    # ~2us, not a full 2MB transfer
                    nc.scalar.dma_start(bvd[:, 0], src[:, 0])
                    nc.scalar.dma_start(bvd[:, 1:], src[:, 1:])
                else:
                    nc.scalar.dma_start(bvd, src)
                bvdm = bvd.bitcast(mybir.dt.float8e4)
                for j in range(CPLD):
                    c = l * CPLD + j
                    for h in range(mh):
                        lhsT = adr_mm[:, c, :, h * P:(h + 1) * P]
                        for q in range(nq):
                            nc.tensor.matmul(
                                psums[h * nq + q],
                                lhsT=lhsT,
                                rhs=bvdm[:, j, :, q * 512:(q + 1) * 512],
                                start=(c == 0),
                                stop=(kreg == 0 and c == ndr - 1),
                                perf_mode=mybir.MatmulPerfMode.DoubleRow,
                            )
            # --- regular segment ---
            for l in range(kreg // cpl):
                bv = bv_pool.tile([P, cpl * nb], mybir.dt.uint8, name="bv")
                nc.scalar.dma_start(bv, bt3[l])
                bvm = bv.bitcast(mybir.dt.float8e3)
                for j in range(cpl):
                    c = l * cpl + j
                    first = c == 0 and ndr == 0
                    last = c == kreg - 1
                    for h in range(mh):
                        for q in range(nq):
                            nc.tensor.matmul(
                                psums[h * nq + q],
                                lhsT=a_sb[:, c, h * P:(h + 1) * P],
                                rhs=bvm[:, j * nb + q * 512:
                                        j * nb + (q + 1) * 512],
                                start=first,
                                stop=last,
                            )
            _epilogue(nc, out_ap, bias_sb, psums, out_pool, mh, nq,
                      psum_all=psum_all)

        if repeat == 1:
            body()
        else:
            # Unroll 2 bodies per hardware-loop iteration: the For_i
            # back-edge is an all-engine barrier (~10us of PE idle: epilogue
            # drain, semaphore storm, then the next B transfer from
            # scratch). Inside the unrolled body the two iterations overlap
            # point-to-point: body 2's DMA kicks issue during body 1's
            # matmul stream and body 2's first matmuls only wait for body
            # 1's per-bank epilogue op.
            assert repeat % 2 == 0, repeat
            with tc.For_i(0, repeat // 2, 1,
                          hint_engines=(mybir.EngineType.PE,
                                        mybir.EngineType.Activation,
                                        mybir.EngineType.DVE,
                                        mybir.EngineType.Pool,
                                        mybir.EngineType.SP)):
                body()
                body()


def _epilogue(nc, out_ap, bias_sb, psums, out_pool, mh, nq, gs_sb=None,
              psum_all=None):
    """(optional global-scale multiply +) bias add + cast to bf16 + store.

    Banks evacuated in the same order the next iteration's chunk-0 matmuls
    will claim them, so the PE restarts after one bank's drain. When
    psum_all is given (all 8 banks as one [P, 4096] f32 tile), banks are
    processed two at a time ([P, 1024] DVE ops amortize the ~300ns
    per-instruction overhead). Stores go on the Sync queue, which idles all
    iteration — keeping the ACT queue kicks-only so the next iteration's B
    loads are not serialized behind store kicks."""
    if psum_all is not None and gs_sb is None:
        for h in range(mh):
            for q2 in range(0, nq, 2):
                ot = out_pool.tile([P, 1024], mybir.dt.bfloat16, name="ot")
                nc.vector.tensor_add(
                    out=ot,
                    in0=psum_all[:, (h * nq + q2) * 512:(h * nq + q2 + 2) * 512],
                    in1=bias_sb[:, q2 * 512:(q2 + 2) * 512],
                )
                nc.sync.dma_start(
                    out_ap[h * P:(h + 1) * P, q2 * 512:(q2 + 2) * 512], ot
                )
        return
    for h in range(mh):
        for q in range(nq):
            ot = out_pool.tile([P, 512], mybir.dt.bfloat16, name="ot")
            if gs_sb is None:
                nc.vector.tensor_add(
                    out=ot,
                    in0=psums[h * nq + q],
                    in1=bias_sb[:, q * 512:(q + 1) * 512],
                )
            else:
                nc.vector.scalar_tensor_tensor(
                    out=ot,
                    in0=psums[h * nq + q],
                    scalar=gs_sb[:, 0:1],
                    in1=bias_sb[:, q * 512:(q + 1) * 512],
                    op0=mybir.AluOpType.mult,
                    op1=mybir.AluOpType.add,
                )
            nc.sync.dma_start(
                out_ap[h * P:(h + 1) * P, q * 512:(q + 1) * 512], ot
            )


def build(kch=KCH, nb=NB, m=M, repeat=1, mode="dequant", ndr=None):
    nc = bacc.Bacc(
        "TRN2",
        target_bir_lowering=False,
        debug=False,
        num_devices=NCORES,
    )
    if mode == "hyb":
        if ndr is None:
            ndr = HYB_NDR
        kreg = kch - 2 * ndr
        cpl = min(8, kreg)
        at = nc.dram_tensor("at", [kreg * P, m], mybir.dt.bfloat16,
                            kind="ExternalInput").ap()
        bt = nc.dram_tensor("bt", [kreg * P // cpl, cpl * nb], mybir.dt.uint8,
                            kind="ExternalInput").ap()
        adr = nc.dram_tensor("adr", [ndr * 256, m], mybir.dt.uint8,
                             kind="ExternalInput").ap()
        btdr = nc.dram_tensor("btdr", [ndr // CPLD * P, CPLD * 2 * nb],
                              mybir.dt.uint8, kind="ExternalInput").ap()
        bias = nc.dram_tensor("bias", [P, nb], mybir.dt.bfloat16,
                              kind="ExternalInput").ap()
        out = nc.dram_tensor("out", [m, nb], mybir.dt.bfloat16,
                             kind="ExternalOutput").ap()
        with tile.TileContext(nc) as tc:
            hyb_body(tc, out, at, bt, adr, btdr, bias, ndr=ndr, kreg=kreg,
                     nb=nb, m=m, repeat=repeat)
        nc.compile()
        return nc
    bt_dt = (mybir.dt.bfloat16 if mode in PRESCALED_MODES else mybir.dt.uint8)
    cpl = _cpl(mode, kch)
    at_dt = mybir.dt.uint8 if mode == "e3a" else mybir.dt.bfloat16
    at = nc.dram_tensor("at", [kch * P, m], at_dt, kind="ExternalInput").ap()
    bt = nc.dram_tensor("bt", [kch * P // cpl, cpl * nb], bt_dt, kind="ExternalInput").ap()
    sbt = nc.dram_tensor("sbt", [kch * 8, nb], mybir.dt.bfloat16, kind="ExternalInput").ap()
    bias = nc.dram_tensor("bias", [P, nb], mybir.dt.bfloat16, kind="ExternalInput").ap()
    gs = (nc.dram_tensor("gs", [P, 1], mybir.dt.float32, kind="ExternalInput").ap()
          if mode == "e3a" else None)
    out = nc.dram_tensor("out", [m, nb], mybir.dt.bfloat16, kind="ExternalOutput").ap()
    with tile.TileContext(nc) as tc:
        tile_body(tc, out, at, bt, sbt, bias, kch=kch, nb=nb, m=m, repeat=repeat,
                  mode=mode, gs_ap=gs)
    nc.compile()
    return nc


def marshal(a, a_scale, a_global_scale, b, b_scale, b_global_scale, bias,
            mode="dequant"):
    """Host-side input prep. Returns per-core in_maps."""
    a = np.asarray(a)
    a_scale = np.asarray(a_scale, np.float32)
    ga = float(np.asarray(a_global_scale, np.float32))
    b = np.asarray(b)
    b_scale = np.asarray(b_scale, np.float32)
    gb = float(np.asarray(b_global_scale, np.float32))
    bias = np.asarray(bias, np.float32)

    if mode == "hyb":
        return _marshal_hyb(a, a_scale, ga, b, b_scale, gb, bias)

    # A side: full dequant (small), fold global scales, transpose to [K, M].
    # In e3 modes B is prescaled by 2x (keeps all e3m4 values normal; min
    # nonzero |bval*bs| = 0.125 < 2^-2 = e3m4 min normal), compensated here.
    a_vals = _FP4[_codes(a)]                                   # [M, K]
    if mode == "e3a":
        # A also e3m4: scale by 2 to stay normal; global scales move to a
        # runtime epilogue multiplier gs = ga*gb/4.
        a_deq = a_vals.reshape(M, K // BLOCK, BLOCK) * (2.0 * a_scale)[..., None]
        at = np.ascontiguousarray(a_deq.reshape(M, K).T).astype(
            ml_dtypes.float8_e3m4).view(np.uint8)
    else:
        a_gl = ga * gb / (2.0 if mode in E3_MODES else 1.0)
        a_deq = a_vals.reshape(M, K // BLOCK, BLOCK) * (a_scale * a_gl)[..., None]
        at = np.ascontiguousarray(a_deq.reshape(M, K).T).astype(ml_dtypes.bfloat16)

    # B side: decode codes to e4m3 value bytes, transpose to [K, N]
    if mode in PRESCALED_MODES:
        bv = _FP4[_codes(b)].reshape(N, K // BLOCK, BLOCK)
        bv = (bv * b_scale.astype(np.float32)[..., None]).reshape(N, K)
        btf = np.ascontiguousarray(bv.T).astype(ml_dtypes.bfloat16)  # [K, N]
    elif mode in E3_MODES:
        bv = _FP4[_codes(b)].reshape(N, K // BLOCK, BLOCK)
        bv = (bv * (2.0 * b_scale.astype(np.float32))[..., None]).reshape(N, K)
        btf = np.ascontiguousarray(bv.T).astype(ml_dtypes.float8_e3m4).view(np.uint8)
    else:
        b_vals_e4m3 = _FP4.astype(ml_dtypes.float8_e4m3)[_codes(b)]  # [N, K]
        btf = np.ascontiguousarray(b_vals_e4m3.T).view(np.uint8)     # [K, N] u8

    # within-chunk k-row permutation: partition p holds original row
    # (p % 8) * 16 + p // 8, so its scale row is (p % 8)
    perm = k_perm(K // P)
    at = np.ascontiguousarray(at[perm])
    btf = btf[perm]
    # pack cpl chunks side by side in the free dim (one DMA per cpl chunks)
    kch = K // P
    cpl = _cpl(mode, kch)
    nfull = btf.shape[1]
    btf = btf.reshape(kch // cpl, cpl, P, nfull).transpose(0, 2, 1, 3)
    sbt_f = np.ascontiguousarray(b_scale.T).astype(ml_dtypes.bfloat16)  # [K/16, N]
    sbt_f = permute_scale_rows(sbt_f, K // P)

    in_maps = []
    for ci in range(NCORES):
        sl = slice(ci * NB, (ci + 1) * NB)
        bias_rep = np.ascontiguousarray(
            np.broadcast_to(bias[None, sl], (P, NB))
        ).astype(ml_dtypes.bfloat16)
        bt_core = np.ascontiguousarray(btf[..., sl]).reshape(
            kch // cpl * P, cpl * NB)
        im = {
            "at": at,
            "bt": bt_core,
            "sbt": np.ascontiguousarray(sbt_f[:, sl]),
            "bias": bias_rep,
        }
        if mode == "e3a":
            im["gs"] = np.full((P, 1), ga * gb / 4.0, np.float32)
        in_maps.append(im)
    return in_maps


def _marshal_hyb(a, a_scale, ga, b, b_scale, gb, bias, ndr=None):
    """Host prep for the DoubleRow/e3m4 K-split. k < ndr*256 goes to the
    DoubleRow segment (both sides e4m3, globals folded into A); the rest to
    the regular segment (A bf16 with g/2, B e3m4 with 2x). No k permutation:
    the DR slot mapping (chunk c, pair i, partition p) -> k = c*256+i*128+p
    is the identity on row-major [K, *] layouts."""
    if ndr is None:
        ndr = HYB_NDR
    g = ga * gb
    kdr = ndr * 256
    kreg = (K - kdr) // P                                  # regular chunks

    a_deq = (_FP4[_codes(a)].reshape(M, K // BLOCK, BLOCK)
             * a_scale[..., None]).reshape(M, K)           # aval*as, [M, K]
    b_deq = (_FP4[_codes(b)].reshape(N, K // BLOCK, BLOCK)
             * b_scale.astype(np.float32)[..., None]).reshape(N, K)

    # DoubleRow segment (e4m3): A carries the global scales. g in [0.25,
    # 2.25] keeps min |A| = 0.125*0.25 = 0.031 >= 2^-6 e4m3 min normal.
    adr = np.ascontiguousarray((a_deq[:, :kdr] * g).T).astype(
        ml_dtypes.float8_e4m3fn).view(np.uint8)            # [kdr, M]
    b8 = np.ascontiguousarray(b_deq[:, :kdr].T).astype(
        ml_dtypes.float8_e4m3fn).view(np.uint8)            # [kdr, N]

    # Regular segment: B*2 keeps e3m4 normal, A absorbs g/2.
    at = np.ascontiguousarray((a_deq[:, kdr:] * (g / 2)).T).astype(
        ml_dtypes.bfloat16)                                # [kreg*128, M]
    breg = np.ascontiguousarray((2.0 * b_deq[:, kdr:]).T).astype(
        ml_dtypes.float8_e3m4).view(np.uint8)              # [kreg*128, N]

    in_maps = []
    for ci in range(NCORES):
        sl = slice(ci * NB, (ci + 1) * NB)
        bias_rep = np.ascontiguousarray(
            np.broadcast_to(np.asarray(bias, np.float32)[None, sl], (P, NB))
        ).astype(ml_dtypes.bfloat16)
        # DR B rows (l,p), cols (j,i,n): k = (l*CPLD+j)*256 + i*128 + p
        btdr = np.ascontiguousarray(
            b8[:, sl].reshape(ndr // CPLD, CPLD, 2, P, NB)
            .transpose(0, 3, 1, 2, 4)
        ).reshape(ndr // CPLD * P, CPLD * 2 * NB)
        cpl = min(8, kreg)
        bt_core = np.ascontiguousarray(
            breg[:, sl].reshape(kreg // cpl, cpl, P, NB).transpose(0, 2, 1, 3)
        ).reshape(kreg // cpl * P, cpl * NB)
        in_maps.append({
            "at": at,
            "bt": bt_core,
            "adr": adr,
            "btdr": btdr,
            "bias": bias_rep,
        })
    return in_maps


_CACHE = {}


MODE = "hyb"


def kernel(a, a_scale, a_global_scale, b, b_scale, b_global_scale, bias):
    in_maps = marshal(a, a_scale, a_global_scale, b, b_scale, b_global_scale,
                      bias, mode=MODE)
    if "nc" not in _CACHE:
        _CACHE["nc"] = build(mode=MODE)
    res = bass_utils.run_bass_kernel_spmd(
        _CACHE["nc"], in_maps, core_ids=list(range(NCORES))
    )
    return np.concatenate([r["out"] for r in res.results], axis=1)



# revision 35
# speedup vs baseline: 1.4235x; 1.0464x over previous
"""NVFP4 block-scaled matmul (A @ B^T + bias) on 8 TRN2 NeuronCores.

Tensor-parallel over N: core i computes out[:, i*2048:(i+1)*2048].

Shipped mode ("hyb"), ~113us/iter steady-state (baseline bf16-prescale
was ~156-160us): a K-split between two host-prescaled pipelines chosen
around the PE's sustained column rate (~263ns per 512-wide matmul; the
2.4GHz burst rate of short runs drops ~20% under the chip's sustained
power management, so the only real lever is streaming fewer columns).

  - DoubleRow segment (HYB_NDR=12 chunks of K=256, 37.5% of K): both
    operands e4m3 (3 mantissa bits), perf_mode=DoubleRow packs 2 fp8
    MACs/cell/cycle, so one [128x512] matmul contracts 256 k at the
    same ~263ns pitch -> half the instructions per k. Costs ~2.8%
    relative error at full K (fails the 2e-2 gate); at f=0.375 the
    blended error is sqrt(f*2.81^2 + (1-f)*1.02^2) ~= 1.91%, measured
    bit-exact between host emulation and HW.
  - Regular segment (40 chunks of K=128): A bf16 (weights) x B e3m4
    (4 mantissa bits, 1 byte/elem wire) mixed-dtype matmuls, ~1.02%
    error. B values are prescaled x2 (min |bval*bs| = 0.125 is
    subnormal in e3m4) with the global scales/2 folded into A.

Per iteration: 96 DR + 320 regular matmuls accumulate into all 8 PSUM
banks (one [128,4096] f32 tile), B streams ~17MB/core on the ACT-kicked
HWDGE queue (2MB row-blocks, first block split so the post-barrier
restart only waits ~1us), epilogue = 4 two-bank [128,1024] DVE
bias-adds + bf16 stores on the otherwise-idle SP queue. The repeat
loop is unrolled 4 bodies per For_i iteration: the back-edge is an
all-engine barrier (~12us: epilogue drain + semaphore storm + cold
B-transfer + HAM re-throttle), while intra-body seams pipeline
point-to-point (~0.6us).

Other modes kept for reference: "prescaled" (bf16 B wire, the old
shipped mode), "e3" (regular-rate e3m4 B only, ~145us), "e3a",
"dequant" (device-side dequant, ~2x slower), "noscale"/"dmaonly"/
"peonly"/"prescaled_ct"/"prescaled_w" diagnostics.
"""

import numpy as np
import ml_dtypes

import concourse.bass as bass
import concourse.mybir as mybir
import concourse.tile as tile
from concourse import bacc
from concourse import bass_utils

P = 128
M, N, K = 256, 16384, 8192
NCORES = 8
NB = N // NCORES          # 2048  per-core N slab
KCH = K // P              # 64    k-chunks of 128
BLOCK = 16                # NVFP4 block size

_FP4 = np.array([0.0, 0.5, 1.0, 1.5, 2.0, 3.0, 4.0, 6.0,
                 -0.0, -0.5, -1.0, -1.5, -2.0, -3.0, -4.0, -6.0], np.float32)

# Host-prescaled bf16 wire format for B (2 bytes/elem)
PRESCALED_MODES = ("prescaled", "prescaled_ct", "prescaled_w", "dmaonly", "peonly")
# Host-prescaled fp8 e3m4 wire format for B (1 byte/elem, 4 mantissa bits).
# "e3": A stays bf16 (mixed-dtype matmul); "e3a": A also e3m4.
E3_MODES = ("e3", "e3a")
STREAM_MODES = PRESCALED_MODES + E3_MODES

# "hyb": the first HYB_NDR chunks of 256 k-values run as fp8e4 DoubleRow
# matmuls (2 fp8 MACs/cell/cycle -> ~2x column rate), the remaining
# 64-2*HYB_NDR chunks of 128 as bf16 x e3m4 regular matmuls. e4m3 has only
# 3 mantissa bits, so the DR slice costs ~2.8% relative error at f=1;
# splitting K keeps total error sqrt(f*2.81^2 + (1-f)*1.02^2) under the
# 2e-2 gate while recovering most of the DoubleRow speed.
HYB_NDR = 12
CPLD = 4          # DR chunks (256 k each) packed per DMA row-block (2MB)


def _cpl(mode, kch):
    # chunks packed side-by-side per DMA row-block: 2MB transfers either way
    return min(8, kch) if mode in E3_MODES else min(4, kch)


def _codes(x_int32: np.ndarray) -> np.ndarray:
    """[rows, K//2] int32 byte values -> [rows, K] uint8 fp4 codes
    (low nibble first, matching the reference)."""
    b = x_int32.astype(np.uint8)
    lo = b & 0xF
    hi = b >> 4
    return np.stack([lo, hi], axis=-1).reshape(b.shape[0], -1)


def permute_scale_rows(sbt: np.ndarray, kch: int) -> np.ndarray:
    """Reorder scale rows for the grouped on-chip replication: within each
    group of G chunks (8*G rows), original row 8*j + pd is stored at
    pd*G + j."""
    G = min(8, kch)
    rows, n = sbt.shape
    return np.ascontiguousarray(
        sbt.reshape(-1, G, 8, n).transpose(0, 2, 1, 3).reshape(rows, n)
    )


def pack_chunks(btf: np.ndarray, kch: int) -> np.ndarray:
    """[kch*P, n] -> [kch//cpl*P, cpl*n]: cpl chunks side by side."""
    cpl = min(4, kch)
    n = btf.shape[1]
    return np.ascontiguousarray(
        btf.reshape(kch // cpl, cpl, P, n).transpose(0, 2, 1, 3)
    ).reshape(kch // cpl * P, cpl * n)


def k_perm(kch: int) -> np.ndarray:
    """Row permutation applied on host: partition p of chunk c holds
    original k-row c*128 + (p % 8)*16 + p//8."""
    p = np.arange(P)
    within = (p % 8) * 16 + p // 8
    return (np.arange(kch)[:, None] * P + within[None, :]).reshape(-1)


def tile_body(tc, out_ap, at_ap, bt_ap, sbt_ap, bias_ap, *, kch=KCH, nb=NB, m=M,
              repeat=1, mode="dequant", gs_ap=None):
    """Per-core kernel body. Shapes:
      at_ap  [kch*128, m]   bf16   A' transposed (dequant, k-major)
      bt_ap  [kch*128, nb]  uint8  e4m3 value bytes of B, k-major
      sbt_ap [kch*8,  nb]   bf16   b_scale transposed (kb-major)
      bias_ap [128, nb]     bf16   bias slab replicated across partitions
      out_ap [m, nb]        bf16
    """
    nc = tc.nc
    assert m % P == 0
    mh = m // P               # m subtiles (2)
    nq = nb // 512            # psum-width quarters (4)
    srows = kch * 8           # total scale rows
    sp = min(srows, P)        # scale slab partition dim
    so = srows // sp

    with (
        tc.tile_pool(name="const", bufs=1) as const,
        tc.tile_pool(name="bv", bufs=6) as bv_pool,
        tc.tile_pool(name="srep", bufs=2) as srep_pool,
        tc.tile_pool(name="bp", bufs=4) as bp_pool,
        tc.tile_pool(name="psum", bufs=1, space="PSUM") as psum_pool,
        tc.tile_pool(name="outp", bufs=8) as out_pool,
    ):
        # Resident tensors (A loaded in 4 pieces so chunk 0 isn't gated on
        # the whole 4MB transfer)
        a_dt = mybir.dt.uint8 if mode == "e3a" else mybir.dt.bfloat16
        a_sb = const.tile([P, kch, m], a_dt, name="a_sb")
        at3 = at_ap.rearrange("(c p) m -> p c m", p=P)
        a_step = max(1, kch // 4)
        for c0 in range(0, kch, a_step):
            c1 = min(kch, c0 + a_step)
            nc.sync.dma_start(a_sb[:, c0:c1], at3[:, c0:c1])
        if mode in STREAM_MODES:
            s_sb = None
        else:
            s_sb = const.tile([sp, so, nb], mybir.dt.bfloat16, name="s_sb")
            nc.sync.dma_start(s_sb, sbt_ap.rearrange("(o p) n -> p o n", p=sp))
        bias_sb = const.tile([P, nb], mybir.dt.bfloat16, name="bias_sb")
        nc.sync.dma_start(bias_sb, bias_ap)
        if gs_ap is not None:
            gs_sb = const.tile([P, 1], mybir.dt.float32, name="gs_sb")
            nc.sync.dma_start(gs_sb, gs_ap)
        else:
            gs_sb = None

        mh = m // P
        nq = nb // 512
        if mode == "prescaled_w":
            psums_w = [
                psum_pool.tile([P, nb], mybir.dt.float32, name=f"psw_{h}")
                for h in range(mh)
            ]
            psums = [psums_w[h][:, q * 512:(q + 1) * 512]
                     for h in range(mh) for q in range(nq)]
        else:
            psums_w = None
            psums = [
                psum_pool.tile([P, 512], mybir.dt.float32, name=f"ps_{h}_{q}")
                for h in range(mh) for q in range(nq)
            ]

        # PE clock warmup: ~5us of dummy matmuls while the first B chunks
        # stream in, so the real stream starts at 2.4GHz (HAM ungates after
        # ~3.4us of sustained PE activity). Chunk 0's start=True resets PSUM,
        # discarding these. Outside the repeat loop: in steady state the PE
        # is already warm and these would be pure per-iteration overhead.
        if mode in ("prescaled", "e3", "e3a"):
            warm = const.tile([P, 512], mybir.dt.bfloat16, name="warm")
            nc.vector.memset(warm, 0.0)
            for w in range(24):
                wi = nc.tensor.matmul(psums[0], lhsT=warm[:, :128], rhs=warm,
                                      start=True, stop=True)
                if w > 0:
                    wi.ins.ldweights = False

        def body():
            _pipeline(tc, out_ap, bt_ap, a_sb, s_sb, bias_sb, psums, psums_w,
                      kch=kch, nb=nb, m=m, sp=sp, mode=mode, gs_sb=gs_sb,
                      bv_pool=bv_pool, srep_pool=srep_pool, bp_pool=bp_pool,
                      out_pool=out_pool)

        if repeat == 1:
            body()
        else:
            with tc.For_i(0, repeat, 1,
                          hint_engines=(mybir.EngineType.PE,
                                        mybir.EngineType.Activation,
                                        mybir.EngineType.DVE,
                                        mybir.EngineType.Pool,
                                        mybir.EngineType.SP)):
                body()


def _pipeline(tc, out_ap, bt_ap, a_sb, s_sb, bias_sb, psums, psums_w, *,
              kch, nb, m, sp,
              bv_pool, srep_pool, bp_pool, out_pool,
              mode="dequant", gs_sb=None):
        nc = tc.nc
        mh = m // P
        nq = nb // 512

        # host packs CPL k-chunks side by side in the free dim:
        # bt row-block l holds chunks l*CPL..(l+1)*CPL at column offsets j*nb
        cpl = _cpl(mode, kch)
        bt3 = bt_ap.rearrange("(l p) n -> l p n", p=P)

        if mode in STREAM_MODES:
            # bt is host-prescaled (bf16 or e3m4); pure DMA + matmul + bias.
            # dmaonly/peonly are perf diagnostics (wrong results).
            e3 = mode in E3_MODES
            bv_dt = mybir.dt.uint8 if e3 else mybir.dt.bfloat16
            if mode == "e3a":
                a_mm = a_sb.bitcast(mybir.dt.float8e3)
            else:
                a_mm = a_sb
            bv0 = None
            for l in range(kch // cpl):
                if mode == "peonly" and bv0 is not None:
                    bv = bv0
                else:
                    bv = bv_pool.tile([P, cpl * nb], bv_dt, name="bv")
                    # B loads ride the ACT-kicked HWDGE queue; the A/bias
                    # prologue stays on the SP queue. Block 0's first chunk
                    # gets its own small kick so the post-barrier matmul
                    # restart waits ~1us instead of a full 2MB transfer.
                    if l == 0:
                        nc.scalar.dma_start(bv[:, :nb], bt3[l][:, :nb])
                        nc.scalar.dma_start(bv[:, nb:], bt3[l][:, nb:])
                    else:
                        nc.scalar.dma_start(bv, bt3[l])
                    bv0 = bv
                bvm = bv.bitcast(mybir.dt.float8e3) if e3 else bv
                for j in range(cpl):
                    c = l * cpl + j
                    first, last = c == 0, c == kch - 1
                    if mode == "dmaonly" and not (first or last):
                        continue
                    if mode == "prescaled_w":
                        for h in range(mh):
                            nc.tensor.matmul(
                                psums_w[h],
                                lhsT=a_mm[:, c, h * P:(h + 1) * P],
                                rhs=bvm[:, j * nb:(j + 1) * nb],
                                start=first,
                                stop=last,
                            )
                        continue
                    for h in range(mh):
                        for q in range(nq):
                            rhs = bvm[:, j * nb + q * 512: j * nb + (q + 1) * 512]
                            if mode == "prescaled_ct":
                                # column-tiled: 4 concurrent 32-col-band
                                # matmuls in separate array quadrants; the
                                # small band weight loads can pull ahead.
                                for b in range(4):
                                    bs = slice(b * 32, (b + 1) * 32)
                                    nc.tensor.matmul(
                                        psums[h * nq + q][bs, :],
                                        lhsT=a_sb[:, c, h * P + b * 32:
                                                  h * P + (b + 1) * 32],
                                        rhs=rhs,
                                        start=first,
                                        stop=last,
                                        tile_position=(0, b * 32),
                                    )
                            else:
                                mi = nc.tensor.matmul(
                                    psums[h * nq + q],
                                    lhsT=a_mm[:, c, h * P:(h + 1) * P],
                                    rhs=rhs,
                                    start=first,
                                    stop=last,
                                )
                                if q > 0:
                                    # q=0 loaded this (c, h) weight tile; the
                                    # other 3 psum quarters reuse it. Walrus
                                    # otherwise emits a ~108ns LDWEIGHTS per
                                    # matmul that row-group-conflicts with the
                                    # in-flight stream (+47ns/matmul).
                                    mi.ins.ldweights = False
            _epilogue(nc, out_ap, bias_sb, psums, out_pool, mh, nq, gs_sb)
            return

        G = min(8, kch)            # chunks per scale-replication group
        for g in range(kch // G):
            # One seed DMA + 4 doubling hops replicate the scales for G
            # chunks at once. Host stores scale rows so that s_sb row
            # pd*G + j (within the group's G*8 rows) = scale row of
            # chunk g*G+j, sub-row pd; after doubling, partition p of
            # column-block j holds scale row 8*(g*G+j) + (p % 8).
            if mode == "noscale":
                srep = None
            else:
                p0 = (8 * G * g) % sp
                o0 = (8 * G * g) // sp
                srep = srep_pool.tile([P, G * nb], mybir.dt.bfloat16, name="srep")
                nc.sync.dma_start(srep[0:8, :], s_sb[p0:p0 + 8 * G, o0, :])
                w = 8
                while w < P:
                    nc.sync.dma_start(srep[w:2 * w], srep[0:w])
                    w *= 2

            for lj in range(G // cpl):
                l = (g * G) // cpl + lj
                # raw e4m3 value bytes for cpl k-chunks in one DMA
                bv = bv_pool.tile([P, cpl * nb], mybir.dt.uint8, name="bv")
                nc.sync.dma_start(bv, bt3[l])
                bv8 = bv.bitcast(mybir.dt.float8e4)

                for jc in range(cpl):
                    c = l * cpl + jc
                    j = c - g * G      # chunk index within the srep group
                    # fp8 -> bf16 convert + scale, produced in 512-wide
                    # quarters so PE starts on quarter 0 while later
                    # quarters convert. ACT ~2.4x GpSimd rate: 3:1 split.
                    bp = bp_pool.tile([P, nb], mybir.dt.bfloat16, name="bp")
                    for q in range(nq):
                        ql = slice(q * 512, (q + 1) * 512)
                        bl = slice(jc * nb + q * 512, jc * nb + (q + 1) * 512)
                        if q == nq - 1:
                            nc.gpsimd.tensor_copy(bp[:, ql], bv8[:, bl])
                        else:
                            nc.scalar.copy(bp[:, ql], bv8[:, bl])
                        s_in = (s_sb[:, 0, q * 512:(q + 1) * 512] if srep is None
                                else srep[:, j * nb + q * 512: j * nb + (q + 1) * 512])
                        nc.vector.tensor_mul(out=bp[:, ql], in0=bp[:, ql], in1=s_in)

                    first = c == 0
                    last = c == kch - 1
                    for h in range(mh):
                        for q in range(nq):
                            nc.tensor.matmul(
                                psums[h * nq + q],
                                lhsT=a_sb[:, c, h * P:(h + 1) * P],
                                rhs=bp[:, q * 512:(q + 1) * 512],
                                start=first,
                                stop=last,
                            )

        _epilogue(nc, out_ap, bias_sb, psums, out_pool, mh, nq)


def hyb_body(tc, out_ap, at_ap, bt_ap, adr_ap, btdr_ap, bias_ap, *,
             ndr, kreg, nb=NB, m=M, repeat=1):
    """K-split kernel: ndr DoubleRow chunks (K=256, both sides e4m3) then
    kreg regular chunks (K=128, bf16 A x e3m4 B), all accumulating into the
    same 8 PSUM banks; bias-add epilogue."""
    nc = tc.nc
    mh = m // P
    nq = nb // 512
    cpl = min(8, kreg)

    with (
        tc.tile_pool(name="const", bufs=1) as const,
        tc.tile_pool(name="bv", bufs=6) as bv_pool,
        tc.tile_pool(name="bvd", bufs=3) as bvd_pool,
        tc.tile_pool(name="psum", bufs=1, space="PSUM") as psum_pool,
        tc.tile_pool(name="outp", bufs=8) as out_pool,
    ):
        # resident A for both segments
        a_sb = const.tile([P, kreg, m], mybir.dt.bfloat16, name="a_sb")
        at3 = at_ap.rearrange("(c p) m -> p c m", p=P)
        a_step = max(1, kreg // 4)
        for c0 in range(0, kreg, a_step):
            c1 = min(kreg, c0 + a_step)
            nc.sync.dma_start(a_sb[:, c0:c1], at3[:, c0:c1])
        adr_sb = const.tile([P, ndr, 2, m], mybir.dt.uint8, name="adr_sb")
        nc.sync.dma_start(adr_sb, adr_ap.rearrange("(c i p) m -> p c i m",
                                                   p=P, i=2))
        adr_mm = adr_sb.bitcast(mybir.dt.float8e4)
        bias_sb = const.tile([P, nb], mybir.dt.bfloat16, name="bias_sb")
        nc.sync.dma_start(bias_sb, bias_ap)

        psum_all = psum_pool.tile([P, mh * nq * 512], mybir.dt.float32,
                                  name="ps_all")
        psums = [psum_all[:, b * 512:(b + 1) * 512] for b in range(mh * nq)]

        warm = const.tile([P, 512], mybir.dt.bfloat16, name="warm")
        nc.vector.memset(warm, 0.0)
        for _ in range(24):
            nc.tensor.matmul(psums[0], lhsT=warm[:, :128], rhs=warm,
                             start=True, stop=True)

        bt3 = bt_ap.rearrange("(l p) n -> l p n", p=P)
        btdr3 = btdr_ap.rearrange("(l p) n -> l p n", p=P)

        def body():
            # --- DoubleRow segment ---
            for l in range(ndr // CPLD):
                bvd = bvd_pool.tile([P, CPLD, 2, nb], mybir.dt.uint8,
                                    name="bvd")
                src = btdr3[l].rearrange("p (j i n) -> p j i n", j=CPLD, i=2)
                if l == 0:
                    # small first kick so the post-barrier restart waits
                    # ~2us, not a full 2MB transfer
                    nc.scalar.dma_start(bvd[:, 0], src[:, 0])
                    nc.scalar.dma_start(bvd[:, 1:], src[:, 1:])
                else:
                    nc.scalar.dma_start(bvd, src)
                bvdm = bvd.bitcast(mybir.dt.float8e4)
                for j in range(CPLD):
                    c = l * CPLD + j
                    for h in range(mh):
                        lhsT = adr_mm[:, c, :, h * P:(h + 1) * P]
                        for q in range(nq):
                            nc.tensor.matmul(
                                psums[h * nq + q],
                                lhsT=lhsT,
                                rhs=bvdm[:, j, :, q * 512:(q + 1) * 512],
                                start=(c == 0),
                                stop=(kreg == 0 and c == ndr - 1),
                                perf_mode=mybir.MatmulPerfMode.DoubleRow,
                            )
            # --- regular segment ---
            for l in range(kreg // cpl):
                bv = bv_pool.tile([P, cpl * nb], mybir.dt.uint8, name="bv")
                nc.scalar.dma_start(bv, bt3[l])
                bvm = bv.bitcast(mybir.dt.float8e3)
                for j in range(cpl):
                    c = l * cpl + j
                    first = c == 0 and ndr == 0
                    last = c == kreg - 1
                    for h in range(mh):
                        for q in range(nq):
                            nc.tensor.matmul(
                                psums[h * nq + q],
                                lhsT=a_sb[:, c, h * P:(h + 1) * P],
                                rhs=bvm[:, j * nb + q * 512:
                                        j * nb + (q + 1) * 512],
                                start=first,
                                stop=last,
                            )
            _epilogue(nc, out_ap, bias_sb, psums, out_pool, mh, nq,
                      psum_all=psum_all)

        if repeat == 1:
            body()
        else:
            # Unroll 4 bodies per hardware-loop iteration: the For_i
            # back-edge is an all-engine barrier (~12us of PE idle: epilogue
            # drain, semaphore storm, then the next B transfer from scratch,
            # plus a HAM cold restart). Inside the unrolled body consecutive
            # iterations overlap point-to-point: the next body's DMA kicks
            # issue during this body's matmul stream (measured seam:
            # ~0.6us) and its first matmuls only wait for the per-bank
            # epilogue op.
            assert repeat % 4 == 0, repeat
            with tc.For_i(0, repeat // 4, 1,
                          hint_engines=(mybir.EngineType.PE,
                                        mybir.EngineType.Activation,
                                        mybir.EngineType.DVE,
                                        mybir.EngineType.Pool,
                                        mybir.EngineType.SP)):
                for _ in range(4):
                    body()


def _epilogue(nc, out_ap, bias_sb, psums, out_pool, mh, nq, gs_sb=None,
              psum_all=None):
    """(optional global-scale multiply +) bias add + cast to bf16 + store.

    Banks evacuated in the same order the next iteration's chunk-0 matmuls
    will claim them, so the PE restarts after one bank's drain. When
    psum_all is given (all 8 banks as one [P, 4096] f32 tile), banks are
    processed two at a time ([P, 1024] DVE ops amortize the ~300ns
    per-instruction overhead). Stores go on the Sync queue, which idles all
    iteration — keeping the ACT queue kicks-only so the next iteration's B
    loads are not serialized behind store kicks."""
    if psum_all is not None and gs_sb is None:
        for h in range(mh):
            for q2 in range(0, nq, 2):
                ot = out_pool.tile([P, 1024], mybir.dt.bfloat16, name="ot")
                nc.vector.tensor_add(
                    out=ot,
                    in0=psum_all[:, (h * nq + q2) * 512:(h * nq + q2 + 2) * 512],
                    in1=bias_sb[:, q2 * 512:(q2 + 2) * 512],
                )
                nc.sync.dma_start(
                    out_ap[h * P:(h + 1) * P, q2 * 512:(q2 + 2) * 512], ot
                )
        return
    for h in range(mh):
        for q in range(nq):
            ot = out_pool.tile([P, 512], mybir.dt.bfloat16, name="ot")
            if gs_sb is None:
                nc.vector.tensor_add(
                    out=ot,
                    in0=psums[h * nq + q],
                    in1=bias_sb[:, q * 512:(q + 1) * 512],
                )
            else:
                nc.vector.scalar_tensor_tensor(
                    out=ot,
                    in0=psums[h * nq + q],
                    scalar=gs_sb[:, 0:1],
                    in1=bias_sb[:, q * 512:(q + 1) * 512],
                    op0=mybir.AluOpType.mult,
                    op1=mybir.AluOpType.add,
                )
            nc.sync.dma_start(
                out_ap[h * P:(h + 1) * P, q * 512:(q + 1) * 512], ot
            )


def build(kch=KCH, nb=NB, m=M, repeat=1, mode="dequant", ndr=None):
    nc = bacc.Bacc(
        "TRN2",
        target_bir_lowering=False,
        debug=False,
        num_devices=NCORES,
    )
    if mode == "hyb":
        if ndr is None:
            ndr = HYB_NDR
        kreg = kch - 2 * ndr
        cpl = min(8, kreg)
        at = nc.dram_tensor("at", [kreg * P, m], mybir.dt.bfloat16,
                            kind="ExternalInput").ap()
        bt = nc.dram_tensor("bt", [kreg * P // cpl, cpl * nb], mybir.dt.uint8,
                            kind="ExternalInput").ap()
        adr = nc.dram_tensor("adr", [ndr * 256, m], mybir.dt.uint8,
                             kind="ExternalInput").ap()
        btdr = nc.dram_tensor("btdr", [ndr // CPLD * P, CPLD * 2 * nb],
                              mybir.dt.uint8, kind="ExternalInput").ap()
        bias = nc.dram_tensor("bias", [P, nb], mybir.dt.bfloat16,
                              kind="ExternalInput").ap()
        out = nc.dram_tensor("out", [m, nb], mybir.dt.bfloat16,
                             kind="ExternalOutput").ap()
        with tile.TileContext(nc) as tc:
            hyb_body(tc, out, at, bt, adr, btdr, bias, ndr=ndr, kreg=kreg,
                     nb=nb, m=m, repeat=repeat)
        nc.compile()
        return nc
    bt_dt = (mybir.dt.bfloat16 if mode in PRESCALED_MODES else mybir.dt.uint8)
    cpl = _cpl(mode, kch)
    at_dt = mybir.dt.uint8 if mode == "e3a" else mybir.dt.bfloat16
    at = nc.dram_tensor("at", [kch * P, m], at_dt, kind="ExternalInput").ap()
    bt = nc.dram_tensor("bt", [kch * P // cpl, cpl * nb], bt_dt, kind="ExternalInput").ap()
    sbt = nc.dram_tensor("sbt", [kch * 8, nb], mybir.dt.bfloat16, kind="ExternalInput").ap()
    bias = nc.dram_tensor("bias", [P, nb], mybir.dt.bfloat16, kind="ExternalInput").ap()
    gs = (nc.dram_tensor("gs", [P, 1], mybir.dt.float32, kind="ExternalInput").ap()
          if mode == "e3a" else None)
    out = nc.dram_tensor("out", [m, nb], mybir.dt.bfloat16, kind="ExternalOutput").ap()
    with tile.TileContext(nc) as tc:
        tile_body(tc, out, at, bt, sbt, bias, kch=kch, nb=nb, m=m, repeat=repeat,
                  mode=mode, gs_ap=gs)
    nc.compile()
    return nc


def marshal(a, a_scale, a_global_scale, b, b_scale, b_global_scale, bias,
            mode="dequant"):
    """Host-side input prep. Returns per-core in_maps."""
    a = np.asarray(a)
    a_scale = np.asarray(a_scale, np.float32)
    ga = float(np.asarray(a_global_scale, np.float32))
    b = np.asarray(b)
    b_scale = np.asarray(b_scale, np.float32)
    gb = float(np.asarray(b_global_scale, np.float32))
    bias = np.asarray(bias, np.float32)

    if mode == "hyb":
        return _marshal_hyb(a, a_scale, ga, b, b_scale, gb, bias)

    # A side: full dequant (small), fold global scales, transpose to [K, M].
    # In e3 modes B is prescaled by 2x (keeps all e3m4 values normal; min
    # nonzero |bval*bs| = 0.125 < 2^-2 = e3m4 min normal), compensated here.
    a_vals = _FP4[_codes(a)]                                   # [M, K]
    if mode == "e3a":
        # A also e3m4: scale by 2 to stay normal; global scales move to a
        # runtime epilogue multiplier gs = ga*gb/4.
        a_deq = a_vals.reshape(M, K // BLOCK, BLOCK) * (2.0 * a_scale)[..., None]
        at = np.ascontiguousarray(a_deq.reshape(M, K).T).astype(
            ml_dtypes.float8_e3m4).view(np.uint8)
    else:
        a_gl = ga * gb / (2.0 if mode in E3_MODES else 1.0)
        a_deq = a_vals.reshape(M, K // BLOCK, BLOCK) * (a_scale * a_gl)[..., None]
        at = np.ascontiguousarray(a_deq.reshape(M, K).T).astype(ml_dtypes.bfloat16)

    # B side: decode codes to e4m3 value bytes, transpose to [K, N]
    if mode in PRESCALED_MODES:
        bv = _FP4[_codes(b)].reshape(N, K // BLOCK, BLOCK)
        bv = (bv * b_scale.astype(np.float32)[..., None]).reshape(N, K)
        btf = np.ascontiguousarray(bv.T).astype(ml_dtypes.bfloat16)  # [K, N]
    elif mode in E3_MODES:
        bv = _FP4[_codes(b)].reshape(N, K // BLOCK, BLOCK)
        bv = (bv * (2.0 * b_scale.astype(np.float32))[..., None]).reshape(N, K)
        btf = np.ascontiguousarray(bv.T).astype(ml_dtypes.float8_e3m4).view(np.uint8)
    else:
        b_vals_e4m3 = _FP4.astype(ml_dtypes.float8_e4m3)[_codes(b)]  # [N, K]
        btf = np.ascontiguousarray(b_vals_e4m3.T).view(np.uint8)     # [K, N] u8

    # within-chunk k-row permutation: partition p holds original row
    # (p % 8) * 16 + p // 8, so its scale row is (p % 8)
    perm = k_perm(K // P)
    at = np.ascontiguousarray(at[perm])
    btf = btf[perm]
    # pack cpl chunks side by side in the free dim (one DMA per cpl chunks)
    kch = K // P
    cpl = _cpl(mode, kch)
    nfull = btf.shape[1]
    btf = btf.reshape(kch // cpl, cpl, P, nfull).transpose(0, 2, 1, 3)
    sbt_f = np.ascontiguousarray(b_scale.T).astype(ml_dtypes.bfloat16)  # [K/16, N]
    sbt_f = permute_scale_rows(sbt_f, K // P)

    in_maps = []
    for ci in range(NCORES):
        sl = slice(ci * NB, (ci + 1) * NB)
        bias_rep = np.ascontiguousarray(
            np.broadcast_to(bias[None, sl], (P, NB))
        ).astype(ml_dtypes.bfloat16)
        bt_core = np.ascontiguousarray(btf[..., sl]).reshape(
            kch // cpl * P, cpl * NB)
        im = {
            "at": at,
            "bt": bt_core,
            "sbt": np.ascontiguousarray(sbt_f[:, sl]),
            "bias": bias_rep,
        }
        if mode == "e3a":
            im["gs"] = np.full((P, 1), ga * gb / 4.0, np.float32)
        in_maps.append(im)
    return in_maps


def _marshal_hyb(a, a_scale, ga, b, b_scale, gb, bias, ndr=None):
    """Host prep for the DoubleRow/e3m4 K-split. k < ndr*256 goes to the
    DoubleRow segment (both sides e4m3, globals folded into A); the rest to
    the regular segment (A bf16 with g/2, B e3m4 with 2x). No k permutation:
    the DR slot mapping (chunk c, pair i, partition p) -> k = c*256+i*128+p
    is the identity on row-major [K, *] layouts."""
    if ndr is None:
        ndr = HYB_NDR
    g = ga * gb
    kdr = ndr * 256
    kreg = (K - kdr) // P                                  # regular chunks

    a_deq = (_FP4[_codes(a)].reshape(M, K // BLOCK, BLOCK)
             * a_scale[..., None]).reshape(M, K)           # aval*as, [M, K]
    b_deq = (_FP4[_codes(b)].reshape(N, K // BLOCK, BLOCK)
             * b_scale.astype(np.float32)[..., None]).reshape(N, K)

    # DoubleRow segment (e4m3): A carries the global scales. g in [0.25,
    # 2.25] keeps min |A| = 0.125*0.25 = 0.031 >= 2^-6 e4m3 min normal.
    adr = np.ascontiguousarray((a_deq[:, :kdr] * g).T).astype(
        ml_dtypes.float8_e4m3fn).view(np.uint8)            # [kdr, M]
    b8 = np.ascontiguousarray(b_deq[:, :kdr].T).astype(
        ml_dtypes.float8_e4m3fn).view(np.uint8)            # [kdr, N]

    # Regular segment: B*2 keeps e3m4 normal, A absorbs g/2.
    at = np.ascontiguousarray((a_deq[:, kdr:] * (g / 2)).T).astype(
        ml_dtypes.bfloat16)                                # [kreg*128, M]
    breg = np.ascontiguousarray((2.0 * b_deq[:, kdr:]).T).astype(
        ml_dtypes.float8_e3m4).view(np.uint8)              # [kreg*128, N]

    in_maps = []
    for ci in range(NCORES):
        sl = slice(ci * NB, (ci + 1) * NB)
        bias_rep = np.ascontiguousarray(
            np.broadcast_to(np.asarray(bias, np.float32)[None, sl], (P, NB))
        ).astype(ml_dtypes.bfloat16)
        # DR B rows (l,p), cols (j,i,n): k = (l*CPLD+j)*256 + i*128 + p
        btdr = np.ascontiguousarray(
            b8[:, sl].reshape(ndr // CPLD, CPLD, 2, P, NB)
            .transpose(0, 3, 1, 2, 4)
        ).reshape(ndr // CPLD * P, CPLD * 2 * NB)
        cpl = min(8, kreg)
        bt_core = np.ascontiguousarray(
            breg[:, sl].reshape(kreg // cpl, cpl, P, NB).transpose(0, 2, 1, 3)
        ).reshape(kreg // cpl * P, cpl * NB)
        in_maps.append({
            "at": at,
            "bt": bt_core,
            "adr": adr,
            "btdr": btdr,
            "bias": bias_rep,
        })
    return in_maps


_CACHE = {}


MODE = "hyb"


def kernel(a, a_scale, a_global_scale, b, b_scale, b_global_scale, bias):
    in_maps = marshal(a, a_scale, a_global_scale, b, b_scale, b_global_scale,
                      bias, mode=MODE)
    if "nc" not in _CACHE:
        _CACHE["nc"] = build(mode=MODE)
    res = bass_utils.run_bass_kernel_spmd(
        _CACHE["nc"], in_maps, core_ids=list(range(NCORES))
    )
    return np.concatenate([r["out"] for r in res.results], axis=1)

